# revision 46
# baseline (speedup 1.0000x reference)
"""Trainium2 Bass kernel for the part-map heatmap-pyramid encoder.

Contract: kernel(part_maps, features) -> (64, 369952) float32.
Data parallel over batch: 8 samples per NeuronCore x 8 cores.

Cost-model reality (CoreSim V1): a DMA occupies its ISSUING engine's queue
for (total_bytes / first_dim_count) * 0.3855 ns (x2 if the contiguous elem
< 512B, min 500ns); there is no shared DMA-bandwidth device. So the design
maximizes the leading AP dim of every transfer (128-row straight copies),
keeps everything fp16 (half bytes, 1 cyc/row matmuls), and spreads DMAs
over the two engines with no compute duties (SP, Pool).

Per-core pipeline:
  1. moments: mom[r, j] = sum_pix P[r,pix] * basis_j(pix) - fp16 TensorE,
     32 chunk matmuls accumulating while pt streams in (2 DMAs, SP+Pool).
  2. sqrt-free coefficient chain on DVE (only a^2, b^2, c^2, cov01 enter
     the final quadratic coefficients) with the c0 tail on Act; PE
     transpose -> coefT fp16. Heat's "+1" is folded into c0.
  3. generation: proj = coefT^T @ [1,y,x,y^2,xy,x^2] fp16 matmuls into
     1536-col PSUM chunks; heat = 1/proj alternating DVE reciprocal / Act
     Reciprocal, fp16 SBUF tiles DMA'd out as [128, N] rows (host
     transposes k-major rows back into the reference layout). The basis
     lives packed in 3 partition-quadrant groups ([70, 7280] as a
     [128, 7280] tensor) so its load is 2 cheap DMAs.
  4. stages 3-5: part-sum via 0/1 selection matmul, rr = 1/(1+sum) on Act,
     broadcast matmul, normalize on DVE, block-diagonal feature matmuls,
     fmap staged fp16 in SBUF ([128, 4*hw] = exactly the HBM row layout).
"""

import numpy as np

BN, NK, NF, HMAP = 64, 16, 64, 64
NCORES = 8
BL = BN // NCORES            # samples per core = 8
ROWS = BL * NK               # partition rows per core = 128
L_INV_SCAL = 0.8
EPS_DIST = 1e-6
EPS_COV = 1e-12

# (h, w, part_depth, (feat_slice_start, feat_slice_end))
STAGES = [(128, 128, NK, (0, 0)), (64, 64, NK, (0, 0)), (32, 32, NK, (0, 0)),
          (16, 16, NK, (4, NK)), (8, 8, 4, (2, 4)), (4, 4, 2, (0, 2))]
HWS = [h * w for (h, w, _, _) in STAGES]          # [16384,4096,1024,256,64,16]
GB_OFF = [0]
for _hw in HWS:
    GB_OFF.append(GB_OFF[-1] + _hw)
GB_TOT = GB_OFF[-1]                                # 21840

# gen-basis packing: 3 column groups on partition quadrants 0/32/64
# (matmul base_partition() only supports those three)
GRP_W = GB_TOT // 3                                # 7280

# full per-sample output offsets (reference layout)
_off = 0
OUT_PH = []
OUT_FM = []
for (h, w, pd, (s0, s1)) in STAGES:
    OUT_PH.append(_off)
    _off += pd * h * w
    if s1 - s0 != 0:
        OUT_FM.append(_off)
        _off += NF * h * w
    else:
        OUT_FM.append(None)
OUT_TOT = _off                                     # 369952

# device heat output: row r = k*8+b, cols = per-row stage blocks
ROFF = GB_OFF                                      # identical layout
HT_COLS = GB_TOT                                   # 21840

# device fmap output [128, 1344]: row = bp*64+n, cols = stage blocks of 4*hw
FOFF = [0, 4 * HWS[3], 4 * (HWS[3] + HWS[4])]
FM_COLS = 4 * (HWS[3] + HWS[4] + HWS[5])           # 1344

# konst packing (fp16 cols): mb [128,160] | sel [128,24] | rep [8,128] |
# ident f16 [128,128]
K_MB, K_SEL, K_REP, K_ID = 0, 160, 184, 312
K_COLS = 440


def _mesh_basis(h, w):
    """Per-pixel basis rows [1, y, x, y^2, x*y, x^2], pixel order i*w+j."""
    y = np.linspace(-1.0, 1.0, h, dtype=np.float64)
    x = np.linspace(-1.0, 1.0, w, dtype=np.float64)
    yy = np.repeat(y, w)
    xx = np.tile(x, h)
    return np.stack([np.ones_like(yy), yy, xx, yy * yy, yy * xx, xx * xx])


def _host_consts():
    # generation basis [6, GB_TOT] packed into quadrants [128, GRP_W]
    gb = np.concatenate([_mesh_basis(h, w) for (h, w, _, _) in STAGES],
                        axis=1).astype(np.float16)
    gbp = np.zeros((128, GRP_W), dtype=np.float16)
    for g in range(3):
        gbp[32 * g:32 * g + 6, :] = gb[:, g * GRP_W:(g + 1) * GRP_W]
    # konst
    konst = np.zeros((128, K_COLS), dtype=np.float16)
    # moment basis: j order [y, x, y^2, x^2, xy] -> [muy, mux, Eyy, Exx, Eyx]
    bm = _mesh_basis(HMAP, HMAP)[[1, 2, 3, 5, 4]]  # [5, 4096]
    for c in range(32):
        konst[:, K_MB + c * 5:K_MB + (c + 1) * 5] = \
            bm[:, c * 128:(c + 1) * 128].T.astype(np.float16)
    for b in range(BL):
        for k in range(NK):
            konst[b, K_REP + k * 8 + b] = 1.0
        for si, sidx in enumerate((3, 4, 5)):
            s0, s1 = STAGES[sidx][3]
            for k in range(s0, s1):
                konst[k * 8 + b, K_SEL + si * 8 + b] = 1.0
    konst[:, K_ID:K_ID + 128] = np.eye(128, dtype=np.float16)
    return gbp, konst


def _host_wf(features_core):
    """Block-diagonal feature weights [128, 12*128] fp16.

    Block (si, g): W[16*b+k, 64*(b-2g)+n] = features[b, k, n] for
    b in {2g, 2g+1} and k in the stage's feature slice, else 0.
    """
    wf = np.zeros((128, 12 * 128), dtype=np.float16)
    for si, sidx in enumerate((3, 4, 5)):
        s0, s1 = STAGES[sidx][3]
        for g in range(4):
            blk = (si * 4 + g) * 128
            for bo in range(2):
                b = 2 * g + bo
                for k in range(s0, s1):
                    wf[k * 8 + b, blk + 64 * bo:blk + 64 * (bo + 1)] = \
                        features_core[b, k, :]
    return wf


_NC_CACHE = {}


def _act_recip(nc, mybir, out, in_, bias=0.0):
    """Reciprocal on the Activation engine: out = 1/(in_ + bias).

    The nc.scalar.activation wrapper refuses Reciprocal (accuracy hygiene
    for generic users); the engine and simulator support it (valid range
    +-[2^-42, 2^42] - our inputs are >= 1), so build the InstActivation
    directly.
    """
    se = nc.scalar
    inputs = [se.lower_ap(in_)]
    for arg in (float(bias), 1.0, 0.0):   # bias, scale, alpha
        inputs.append(mybir.ImmediateValue(dtype=mybir.dt.float32, value=arg))
    return se.add_instruction(
        mybir.InstActivation(
            name=se.bass.get_next_instruction_name(),
            func=mybir.ActivationFunctionType.Reciprocal,
            ins=inputs,
            outs=[se.lower_ap(out)],
        ))


def _build():
    import contextlib
    import concourse.bacc as bacc
    import concourse.tile as tile
    from concourse import mybir

    f32 = mybir.dt.float32
    f16 = mybir.dt.float16
    AT = mybir.AluOpType

    nc = bacc.Bacc("TRN2", target_bir_lowering=False, debug=False)
    pt = nc.declare_dram_parameter("pt", [128, 4096], f16, isOutput=False)
    gb = nc.declare_dram_parameter("gb", [128, GRP_W], f16, isOutput=False)
    kon = nc.declare_dram_parameter("kon", [128, K_COLS], f16, isOutput=False)
    wf = nc.declare_dram_parameter("wf", [128, 12 * 128], f16, isOutput=False)
    outh = nc.declare_dram_parameter("outh", [128, HT_COLS], f16, isOutput=True)
    outf = nc.declare_dram_parameter("outf", [128, FM_COLS], f16, isOutput=True)

    with tile.TileContext(nc) as tc:
        ctx = contextlib.ExitStack()
        with ctx:
            consts = ctx.enter_context(tc.tile_pool(name="consts", bufs=1))
            sm = ctx.enter_context(tc.tile_pool(name="sm", bufs=1))
            hp = ctx.enter_context(tc.tile_pool(name="hp", bufs=4))
            pgen = ctx.enter_context(
                tc.tile_pool(name="pgen", bufs=2, space="PSUM"))
            pmisc = ctx.enter_context(
                tc.tile_pool(name="pmisc", bufs=2, space="PSUM"))

            # ---- inputs, split across SP and Pool queues ----
            skon = consts.tile([128, K_COLS], f16)
            nc.sync.dma_start(out=skon, in_=kon[:, :])
            pts = consts.tile([128, 4096], f16)
            nc.sync.dma_start(out=pts[:, 0:1024], in_=pt[:, 0:1024])
            nc.scalar.dma_start(out=pts[:, 1024:2560], in_=pt[:, 1024:2560])
            nc.gpsimd.dma_start(out=pts[:, 2560:4096], in_=pt[:, 2560:4096])
            gbs = consts.tile([128, GRP_W], f16)
            nc.sync.dma_start(out=gbs[:, 0:1820], in_=gb[:, 0:1820])
            nc.sync.dma_start(out=gbs[:, 1820:GRP_W // 2],
                              in_=gb[:, 1820:GRP_W // 2])
            nc.gpsimd.dma_start(out=gbs[:, GRP_W // 2:GRP_W],
                                in_=gb[:, GRP_W // 2:GRP_W])
            swf = consts.tile([128, 12 * 128], f16)
            nc.gpsimd.dma_start(out=swf, in_=wf[:, :])
            smbs = skon[:, K_MB:K_MB + 160]
            ssel = skon[:, K_SEL:K_SEL + 24]
            srep = skon[0:8, K_REP:K_REP + 128]
            sident = skon[:, K_ID:K_ID + 128]

            # Act table warm-up (one Reciprocal table load, overlaps head)
            warm = sm.tile([128, 1], f32, tag="warm")
            _act_recip(nc, mybir, warm, skon[:, 0:1])

            # PE p-state warm-up: the clock ramps for 3us from the FIRST PE
            # instruction; a dummy matmul at t~0.3 gets the stream matmuls
            # to full clock ~4us earlier than the moment phase would.
            zt = sm.tile([128, 1], f16, tag="zt")
            nc.vector.memset(zt, 1.0)
            pdum = pmisc.tile([1, 1], f32, tag="pm", name="pdum")
            nc.tensor.matmul(pdum, lhsT=zt, rhs=zt, start=True, stop=True)

            # ---- phase 1: moments (fp16 in, fp32 accum) ----
            psmom = pmisc.tile([128, 8], f32, tag="pm")
            for c in range(32):
                nc.tensor.matmul(
                    psmom[:, 0:5],
                    lhsT=pts[:, c * 128:(c + 1) * 128],
                    rhs=smbs[:, c * 5:(c + 1) * 5],
                    start=(c == 0),
                    stop=(c == 31),
                )

            # ---- phase 2: per-row quadratic-form coefficients ----
            def t(cols, tag):
                return sm.tile([128, cols], f32, tag=tag, name=tag)

            mom = t(5, "mom")
            nc.vector.tensor_copy(out=mom, in_=psmom[:, 0:5])
            pp = t(2, "pp")
            nc.vector.tensor_scalar(out=pp, in0=mom[:, 0:2], scalar1=-1.0,
                                    scalar2=EPS_DIST, op0=AT.mult, op1=AT.add)
            sq2 = t(2, "sq2")
            nc.vector.tensor_tensor(out=sq2, in0=pp, in1=pp, op=AT.mult)
            pxy = t(1, "pxy")
            nc.vector.tensor_tensor(out=pxy, in0=pp[:, 0:1], in1=pp[:, 1:2],
                                    op=AT.mult)
            musq = t(2, "musq")
            nc.vector.tensor_tensor(out=musq, in0=mom[:, 0:2], in1=mom[:, 0:2],
                                    op=AT.mult)
            mxy = t(1, "mxy")
            nc.vector.tensor_tensor(out=mxy, in0=mom[:, 0:1], in1=mom[:, 1:2],
                                    op=AT.mult)
            cov2 = t(2, "cov2")       # [cov00, cov11]
            nc.vector.tensor_tensor(out=cov2, in0=mom[:, 2:4], in1=musq,
                                    op=AT.subtract)
            cov01 = t(1, "cov01")
            nc.vector.tensor_tensor(out=cov01, in0=mom[:, 4:5], in1=mxy,
                                    op=AT.subtract)
            ra = t(1, "ra")           # 1/cov00 (+1e-12 is sub-ulp here)
            nc.vector.reciprocal_approx_fast(out=ra, in_=cov2[:, 0:1])
            c01sq = t(1, "c01sq")
            nc.vector.tensor_tensor(out=c01sq, in0=cov01, in1=cov01, op=AT.mult)
            b2 = t(1, "b2")
            nc.vector.tensor_tensor(out=b2, in0=c01sq, in1=ra, op=AT.mult)
            c2v = t(1, "c2v")         # cov11 - b2
            nc.vector.scalar_tensor_tensor(out=c2v, in0=b2, scalar=-1.0,
                                           in1=cov2[:, 1:2], op0=AT.mult,
                                           op1=AT.add)
            ac = t(1, "ac")
            nc.vector.tensor_tensor(out=ac, in0=cov2[:, 0:1], in1=c2v,
                                    op=AT.mult)
            rq = t(1, "rq")           # 1/(a2*c2)
            nc.vector.reciprocal_approx_fast(out=rq, in_=ac)
            tBC = t(1, "tBC")
            nc.vector.tensor_tensor(out=tBC, in0=b2, in1=c2v, op=AT.add)

            Q = L_INV_SCAL * L_INV_SCAL
            coef = sm.tile([128, 6], f32, tag="coef")
            nc.vector.scalar_tensor_tensor(out=coef[:, 3:4], in0=tBC, scalar=Q,
                                           in1=rq, op0=AT.mult, op1=AT.mult)
            nc.vector.scalar_tensor_tensor(out=coef[:, 4:5], in0=cov01,
                                           scalar=-2.0 * Q, in1=rq,
                                           op0=AT.mult, op1=AT.mult)
            nc.vector.scalar_tensor_tensor(out=coef[:, 5:6], in0=cov2[:, 0:1],
                                           scalar=Q, in1=rq, op0=AT.mult,
                                           op1=AT.mult)
            # c1 = 2A*py + B*px ; c2 = B*py + 2C*px   (DVE)
            t4 = t(1, "t4")
            nc.vector.scalar_tensor_tensor(out=t4, in0=coef[:, 3:4], scalar=2.0,
                                           in1=pp[:, 0:1], op0=AT.mult,
                                           op1=AT.mult)
            t5 = t(1, "t5")
            nc.vector.tensor_tensor(out=t5, in0=coef[:, 4:5], in1=pp[:, 1:2],
                                    op=AT.mult)
            nc.vector.tensor_tensor(out=coef[:, 1:2], in0=t4, in1=t5, op=AT.add)
            t6 = t(1, "t6")
            nc.vector.tensor_tensor(out=t6, in0=coef[:, 4:5], in1=pp[:, 0:1],
                                    op=AT.mult)
            t7 = t(1, "t7")
            nc.vector.scalar_tensor_tensor(out=t7, in0=coef[:, 5:6], scalar=2.0,
                                           in1=pp[:, 1:2], op0=AT.mult,
                                           op1=AT.mult)
            nc.vector.tensor_tensor(out=coef[:, 2:3], in0=t6, in1=t7, op=AT.add)
            # c0 = 1 + A*py^2 + B*py*px + C*px^2 as 3 chained Act ops
            # (Identity computes scale*x + bias with per-partition APs)
            a1 = t(1, "a1")
            nc.scalar.activation(a1, sq2[:, 0:1], mybir.ActivationFunctionType
                                 .Identity, bias=1.0, scale=coef[:, 3:4])
            a2t = t(1, "a2t")
            nc.scalar.activation(a2t, sq2[:, 1:2], mybir.ActivationFunctionType
                                 .Identity, bias=a1, scale=coef[:, 5:6])
            nc.scalar.activation(coef[:, 0:1], pxy,
                                 mybir.ActivationFunctionType.Identity,
                                 bias=a2t, scale=coef[:, 4:5])

            # coef^T as a regular fp16 matmul (lhsT=coef16, rhs=ident16),
            # written to all three partition quadrants (matmul lhsT/rhs share
            # base partition, so each basis group needs its own coefT copy).
            # Group 0 first - the stream starts on it; groups 1/2 follow
            # during the first chunks (not needed until basis col 7280).
            coef16 = sm.tile([128, 6], f16, tag="coef16")
            nc.vector.tensor_copy(out=coef16, in_=coef)
            coefTq = sm.tile([70, 128], f16, tag="coefTq")

            def rep_coef(g):
                pstq = pmisc.tile([70, 128], f32, tag="pm", name=f"pstq{g}")
                nc.tensor.matmul(pstq[32 * g:32 * g + 6, :], lhsT=coef16,
                                 rhs=sident, start=True, stop=True)
                nc.vector.tensor_copy(out=coefTq[32 * g:32 * g + 6, :],
                                      in_=pstq[32 * g:32 * g + 6, :])

            rep_coef(0)
            rep_coef(1)
            rep_coef(2)

            # ---- generation helpers ----
            def gen_mm(pg, pcol, glo, ghi):
                """Matmuls filling pg[:, pcol:...] with proj for global basis
                cols [glo, ghi), splitting at basis quadrant boundaries.
                Caller keeps each call within one 512-col PSUM bank span."""
                while glo < ghi:
                    g = glo // GRP_W
                    n = min(ghi, (g + 1) * GRP_W) - glo
                    nc.tensor.matmul(
                        pg[:, pcol:pcol + n],
                        lhsT=coefTq[32 * g:32 * g + 6, :],
                        rhs=gbs[32 * g:32 * g + 6,
                                glo - g * GRP_W:glo - g * GRP_W + n],
                        start=True, stop=True)
                    pcol += n
                    glo += n

            recip_sched = []   # "D"/"A" consumed per psum chunk

            def recip(dst, pg):
                if recip_sched.pop(0) == "D":
                    with nc.allow_low_precision(reason="fp16 heat by design"):
                        nc.vector.reciprocal(out=dst, in_=pg)
                else:
                    _act_recip(nc, mybir, dst, pg)

            def gen_chunk(glo, cols, dst, dcol):
                """One PSUM chunk [128, cols] -> reciprocal -> dst fp16."""
                pg = pgen.tile([128, cols], f32, tag="ps", name="ps")
                for q in range(0, cols, 512):
                    gen_mm(pg, q, glo + q, glo + min(q + 512, cols))
                recip(dst[:, dcol:dcol + cols], pg)

            # ---- stages 3-5: heat resident in SBUF ----
            H = {}
            recip_sched += ["D", "A", "A"]
            for sidx in (3, 4, 5):
                hw = HWS[sidx]
                Hs = sm.tile([128, hw], f16, tag=f"H{sidx}", name=f"H{sidx}")
                pg = pmisc.tile([128, hw], f32, tag="pm", name=f"pgH{sidx}")
                gen_mm(pg, 0, GB_OFF[sidx], GB_OFF[sidx] + hw)
                recip(Hs, pg)
                H[sidx] = Hs

            # part_heat outputs, stages 3-5
            nc.sync.dma_start(out=outh[:, ROFF[3]:ROFF[3] + HWS[3]], in_=H[3])
            nc.gpsimd.dma_start(out=outh[0:32, ROFF[4]:ROFF[4] + HWS[4]],
                                in_=H[4][0:32, :])
            nc.gpsimd.dma_start(out=outh[0:16, ROFF[5]:ROFF[5] + HWS[5]],
                                in_=H[5][0:16, :])

            # part-sums for the fmap normalizers (PE, ready early)
            pss = {}
            for si, sidx in enumerate((3, 4, 5)):
                ps = pmisc.tile([8, HWS[sidx]], f32, tag="pm",
                                name=f"pss{sidx}")
                nc.tensor.matmul(ps, lhsT=ssel[:, si * 8:(si + 1) * 8],
                                 rhs=H[sidx], start=True, stop=True)
                pss[sidx] = ps

            # fmap chains (emitted mid-stream so all engines stay busy)
            def fmap_chains():
                for si, sidx in enumerate((3, 4, 5)):
                    hw = HWS[sidx]
                    rr = sm.tile([8, hw], f16, tag=f"rr{sidx}",
                                 name=f"rr{sidx}")
                    _act_recip(nc, mybir, rr, pss[sidx], bias=1.0)
                    psR = pmisc.tile([128, hw], f32, tag="pm",
                                     name=f"psR{sidx}")
                    nc.tensor.matmul(psR, lhsT=srep, rhs=rr, start=True,
                                     stop=True)
                    Hn = sm.tile([128, hw], f16, tag=f"Hn{sidx}",
                                 name=f"Hn{sidx}")
                    nc.vector.tensor_tensor(out=Hn, in0=H[sidx], in1=psR,
                                            op=AT.mult)
                    fm = sm.tile([128, 4 * hw], f16, tag=f"fm{sidx}",
                                 name=f"fm{sidx}")
                    for half in range(2):
                        pf = pmisc.tile([128, 2 * hw], f32, tag="pm",
                                        name="pf")
                        for bo in range(2):
                            g = half * 2 + bo
                            nc.tensor.matmul(
                                pf[:, bo * hw:(bo + 1) * hw],
                                lhsT=swf[:, (si * 4 + g) * 128:
                                         (si * 4 + g + 1) * 128],
                                rhs=Hn, start=True, stop=True)
                        dst = fm[:, half * 2 * hw:(half + 1) * 2 * hw]
                        if half == 0:
                            nc.scalar.copy(dst, pf)
                        else:
                            nc.vector.tensor_copy(out=dst, in_=pf)
                    eng = nc.gpsimd if sidx != 4 else nc.sync
                    eng.dma_start(out=outf[:, FOFF[si]:FOFF[si] + 4 * hw],
                                  in_=fm)

            # ---- stage 0-2 streaming: 1536-col chunks, 1 DMA per chunk ----
            CH_SCHED = ["D" if i % 2 else "A" for i in range(15)]
            DMA_ENG = [0]
            hold = [fmap_chains]

            def stream_stage(sidx):
                hw = HWS[sidx]
                base = GB_OFF[sidx]
                n0 = 0
                while n0 < hw:
                    cols = min(1536, hw - n0)
                    recip_sched.append(CH_SCHED.pop(0))
                    ht = hp.tile([128, cols], f16, tag="ht", name="ht")
                    gen_chunk(base + n0, cols, ht, 0)
                    eng = (nc.sync, nc.gpsimd)[DMA_ENG[0] % 2]
                    DMA_ENG[0] += 1
                    eng.dma_start(
                        out=outh[:, ROFF[sidx] + n0:ROFF[sidx] + n0 + cols],
                        in_=ht)
                    n0 += cols
                    if hold[0] is not None and DMA_ENG[0] >= 2:
                        fn = hold[0]
                        hold[0] = None
                        fn()

            stream_stage(0)
            stream_stage(1)
            # stage 2 (the 1024-col tail): halves on BOTH recip engines and
            # both DMA queues so the pipeline drain is ~2x shorter.
            pgl = pgen.tile([128, 1024], f32, tag="ps", name="ps")
            for q in (0, 512):
                gen_mm(pgl, q, GB_OFF[2] + q, GB_OFF[2] + q + 512)
            htl = hp.tile([128, 1024], f16, tag="ht", name="ht")
            with nc.allow_low_precision(reason="fp16 heat by design"):
                nc.vector.reciprocal(out=htl[:, 0:512], in_=pgl[:, 0:512])
            _act_recip(nc, mybir, htl[:, 512:1024], pgl[:, 512:1024])
            nc.sync.dma_start(out=outh[:, ROFF[2]:ROFF[2] + 512],
                              in_=htl[:, 0:512])
            nc.gpsimd.dma_start(out=outh[:, ROFF[2] + 512:ROFF[2] + 1024],
                                in_=htl[:, 512:1024])

    nc.compile()
    return nc


def _get_nc():
    if "nc" not in _NC_CACHE:
        _NC_CACHE["nc"] = _build()
    return _NC_CACHE["nc"]


def _in_maps(part_maps, features):
    part_maps = np.asarray(part_maps, dtype=np.float32)
    features = np.asarray(features, dtype=np.float32)
    gbp, konst = _host_consts()
    in_maps = []
    for core in range(NCORES):
        pm = part_maps[core * BL:(core + 1) * BL]          # [8, 16, 64, 64]
        # k-major row order: row r = k*8 + b; pt[p, c*128+r] = P[r, c*128+p]
        P = pm.transpose(1, 0, 2, 3).reshape(ROWS, HMAP * HMAP)
        pt = np.ascontiguousarray(
            P.reshape(128, 32, 128).transpose(2, 1, 0).reshape(128, 4096)
        ).astype(np.float16)
        wfc = _host_wf(features[core * BL:(core + 1) * BL])
        in_maps.append({"pt": pt, "gb": gbp, "kon": konst, "wf": wfc})
    return in_maps


def _assemble(oh, of):
    """Merge device-layout fp16 outputs into the reference layout (fp32)."""
    full = np.empty((BL, OUT_TOT), np.float32)
    fm_i = 0
    for s in range(6):
        pd, hw = STAGES[s][2], HWS[s]
        blk = oh[:pd * BL, ROFF[s]:ROFF[s] + hw].astype(np.float32)
        full[:, OUT_PH[s]:OUT_PH[s] + pd * hw] = \
            blk.reshape(pd, BL, hw).transpose(1, 0, 2).reshape(BL, pd * hw)
        if OUT_FM[s] is not None:
            fmb = of[:, FOFF[fm_i]:FOFF[fm_i] + 4 * hw].astype(np.float32)
            # fmb[bp*64+n, g*hw+f] -> full[2g+bp, n*hw+f]
            arr = fmb.reshape(2, NF, 4, hw).transpose(2, 0, 1, 3)
            full[:, OUT_FM[s]:OUT_FM[s] + NF * hw] = \
                arr.reshape(BL, NF * hw)
            fm_i += 1
    return full


def _run(part_maps, features, trace=False):
    from concourse.bass_utils import run_bass_kernel_spmd
    nc = _get_nc()
    res = run_bass_kernel_spmd(nc, _in_maps(part_maps, features),
                               list(range(NCORES)), trace=trace)
    outs = [_assemble(res.results[i]["outh"], res.results[i]["outf"])
            for i in range(NCORES)]
    return np.concatenate(outs, axis=0), res


def kernel(part_maps, features):
    out, _ = _run(part_maps, features, trace=False)
    return out


# revision 47
# speedup vs baseline: 1.0409x; 1.0409x over previous
"""Trainium2 Bass kernel for the part-map heatmap-pyramid encoder.

Contract: kernel(part_maps, features) -> (64, 369952) float32.
Data parallel over batch: 8 samples per NeuronCore x 8 cores.

Cost-model reality (CoreSim V1): a DMA occupies its ISSUING engine's queue
for (total_bytes / first_dim_count) * 0.3855 ns (x2 if the contiguous elem
< 512B, min 500ns); there is no shared DMA-bandwidth device. So the design
maximizes the leading AP dim of every transfer (128-row straight copies),
keeps everything fp16 (half bytes, 1 cyc/row matmuls), and spreads DMAs
over the two engines with no compute duties (SP, Pool).

Per-core pipeline:
  1. moments: mom[r, j] = sum_pix P[r,pix] * basis_j(pix) - fp16 TensorE,
     32 chunk matmuls accumulating while pt streams in (2 DMAs, SP+Pool).
  2. sqrt-free coefficient chain on DVE (only a^2, b^2, c^2, cov01 enter
     the final quadratic coefficients) with the c0 tail on Act; PE
     transpose -> coefT fp16. Heat's "+1" is folded into c0.
  3. generation: proj = coefT^T @ [1,y,x,y^2,xy,x^2] fp16 matmuls into
     1536-col PSUM chunks; heat = 1/proj alternating DVE reciprocal / Act
     Reciprocal, fp16 SBUF tiles DMA'd out as [128, N] rows (host
     transposes k-major rows back into the reference layout). The basis
     lives packed in 3 partition-quadrant groups ([70, 7280] as a
     [128, 7280] tensor) so its load is 2 cheap DMAs.
  4. stages 3-5: part-sum via 0/1 selection matmul, rr = 1/(1+sum) on Act,
     broadcast matmul, normalize on DVE, block-diagonal feature matmuls,
     fmap staged fp16 in SBUF ([128, 4*hw] = exactly the HBM row layout).
"""

import numpy as np

BN, NK, NF, HMAP = 64, 16, 64, 64
NCORES = 8
BL = BN // NCORES            # samples per core = 8
ROWS = BL * NK               # partition rows per core = 128
L_INV_SCAL = 0.8
EPS_DIST = 1e-6
EPS_COV = 1e-12

# (h, w, part_depth, (feat_slice_start, feat_slice_end))
STAGES = [(128, 128, NK, (0, 0)), (64, 64, NK, (0, 0)), (32, 32, NK, (0, 0)),
          (16, 16, NK, (4, NK)), (8, 8, 4, (2, 4)), (4, 4, 2, (0, 2))]
HWS = [h * w for (h, w, _, _) in STAGES]          # [16384,4096,1024,256,64,16]
GB_OFF = [0]
for _hw in HWS:
    GB_OFF.append(GB_OFF[-1] + _hw)
GB_TOT = GB_OFF[-1]                                # 21840

# gen-basis packing: 3 column groups on partition quadrants 0/32/64
# (matmul base_partition() only supports those three)
GRP_W = GB_TOT // 3                                # 7280

# full per-sample output offsets (reference layout)
_off = 0
OUT_PH = []
OUT_FM = []
for (h, w, pd, (s0, s1)) in STAGES:
    OUT_PH.append(_off)
    _off += pd * h * w
    if s1 - s0 != 0:
        OUT_FM.append(_off)
        _off += NF * h * w
    else:
        OUT_FM.append(None)
OUT_TOT = _off                                     # 369952

# device heat output: row r = k*8+b, cols = per-row stage blocks
ROFF = GB_OFF                                      # identical layout
HT_COLS = GB_TOT                                   # 21840

# device fmap output [128, 1344]: row = bp*64+n, cols = stage blocks of 4*hw
FOFF = [0, 4 * HWS[3], 4 * (HWS[3] + HWS[4])]
FM_COLS = 4 * (HWS[3] + HWS[4] + HWS[5])           # 1344

# konst packing (fp16 cols): mb [128,160] | sel [128,24] | rep [8,128] |
# ident f16 [128,128]
K_MB, K_SEL, K_REP, K_ID = 0, 160, 184, 312
K_COLS = 440


def _mesh_basis(h, w):
    """Per-pixel basis rows [1, y, x, y^2, x*y, x^2], pixel order i*w+j."""
    y = np.linspace(-1.0, 1.0, h, dtype=np.float64)
    x = np.linspace(-1.0, 1.0, w, dtype=np.float64)
    yy = np.repeat(y, w)
    xx = np.tile(x, h)
    return np.stack([np.ones_like(yy), yy, xx, yy * yy, yy * xx, xx * xx])


def _host_consts():
    # generation basis [6, GB_TOT] packed into quadrants [128, GRP_W]
    gb = np.concatenate([_mesh_basis(h, w) for (h, w, _, _) in STAGES],
                        axis=1).astype(np.float16)
    gbp = np.zeros((128, GRP_W), dtype=np.float16)
    for g in range(3):
        gbp[32 * g:32 * g + 6, :] = gb[:, g * GRP_W:(g + 1) * GRP_W]
    # konst
    konst = np.zeros((128, K_COLS), dtype=np.float16)
    # moment basis: j order [y, x, y^2, x^2, xy] -> [muy, mux, Eyy, Exx, Eyx]
    bm = _mesh_basis(HMAP, HMAP)[[1, 2, 3, 5, 4]]  # [5, 4096]
    for c in range(32):
        konst[:, K_MB + c * 5:K_MB + (c + 1) * 5] = \
            bm[:, c * 128:(c + 1) * 128].T.astype(np.float16)
    for b in range(BL):
        for k in range(NK):
            konst[b, K_REP + k * 8 + b] = 1.0
        for si, sidx in enumerate((3, 4, 5)):
            s0, s1 = STAGES[sidx][3]
            for k in range(s0, s1):
                konst[k * 8 + b, K_SEL + si * 8 + b] = 1.0
    konst[:, K_ID:K_ID + 128] = np.eye(128, dtype=np.float16)
    return gbp, konst


def _host_wf(features_core):
    """Block-diagonal feature weights [128, 12*128] fp16.

    Block (si, g): W[16*b+k, 64*(b-2g)+n] = features[b, k, n] for
    b in {2g, 2g+1} and k in the stage's feature slice, else 0.
    """
    wf = np.zeros((128, 12 * 128), dtype=np.float16)
    for si, sidx in enumerate((3, 4, 5)):
        s0, s1 = STAGES[sidx][3]
        for g in range(4):
            blk = (si * 4 + g) * 128
            for bo in range(2):
                b = 2 * g + bo
                for k in range(s0, s1):
                    wf[k * 8 + b, blk + 64 * bo:blk + 64 * (bo + 1)] = \
                        features_core[b, k, :]
    return wf


_NC_CACHE = {}


def _act_recip(nc, mybir, out, in_, bias=0.0):
    """Reciprocal on the Activation engine: out = 1/(in_ + bias).

    The nc.scalar.activation wrapper refuses Reciprocal (accuracy hygiene
    for generic users); the engine and simulator support it (valid range
    +-[2^-42, 2^42] - our inputs are >= 1), so build the InstActivation
    directly.
    """
    se = nc.scalar
    inputs = [se.lower_ap(in_)]
    for arg in (float(bias), 1.0, 0.0):   # bias, scale, alpha
        inputs.append(mybir.ImmediateValue(dtype=mybir.dt.float32, value=arg))
    return se.add_instruction(
        mybir.InstActivation(
            name=se.bass.get_next_instruction_name(),
            func=mybir.ActivationFunctionType.Reciprocal,
            ins=inputs,
            outs=[se.lower_ap(out)],
        ))


def _build():
    import contextlib
    import concourse.bacc as bacc
    import concourse.tile as tile
    from concourse import mybir

    f32 = mybir.dt.float32
    f16 = mybir.dt.float16
    AT = mybir.AluOpType

    nc = bacc.Bacc("TRN2", target_bir_lowering=False, debug=False)
    pt = nc.declare_dram_parameter("pt", [128, 4096], f16, isOutput=False)
    gb = nc.declare_dram_parameter("gb", [128, GRP_W], f16, isOutput=False)
    kon = nc.declare_dram_parameter("kon", [128, K_COLS], f16, isOutput=False)
    wf = nc.declare_dram_parameter("wf", [128, 12 * 128], f16, isOutput=False)
    outh = nc.declare_dram_parameter("outh", [128, HT_COLS], f16, isOutput=True)
    outf = nc.declare_dram_parameter("outf", [128, FM_COLS], f16, isOutput=True)

    with tile.TileContext(nc) as tc:
        ctx = contextlib.ExitStack()
        with ctx:
            consts = ctx.enter_context(tc.tile_pool(name="consts", bufs=1))
            sm = ctx.enter_context(tc.tile_pool(name="sm", bufs=1))
            hp = ctx.enter_context(tc.tile_pool(name="hp", bufs=4))
            pgen = ctx.enter_context(
                tc.tile_pool(name="pgen", bufs=2, space="PSUM"))
            pmisc = ctx.enter_context(
                tc.tile_pool(name="pmisc", bufs=2, space="PSUM"))

            # ---- inputs, split across SP and Pool queues ----
            skon = consts.tile([128, K_COLS], f16)
            nc.sync.dma_start(out=skon, in_=kon[:, :])
            pts = consts.tile([128, 4096], f16)
            nc.sync.dma_start(out=pts[:, 0:1024], in_=pt[:, 0:1024])
            nc.scalar.dma_start(out=pts[:, 1024:2560], in_=pt[:, 1024:2560])
            nc.gpsimd.dma_start(out=pts[:, 2560:4096], in_=pt[:, 2560:4096])
            gbs = consts.tile([128, GRP_W], f16)
            nc.sync.dma_start(out=gbs[:, 0:1820], in_=gb[:, 0:1820])
            nc.sync.dma_start(out=gbs[:, 1820:GRP_W // 2],
                              in_=gb[:, 1820:GRP_W // 2])
            nc.gpsimd.dma_start(out=gbs[:, GRP_W // 2:GRP_W],
                                in_=gb[:, GRP_W // 2:GRP_W])
            swf = consts.tile([128, 12 * 128], f16)
            nc.gpsimd.dma_start(out=swf, in_=wf[:, :])
            smbs = skon[:, K_MB:K_MB + 160]
            ssel = skon[:, K_SEL:K_SEL + 24]
            srep = skon[0:8, K_REP:K_REP + 128]
            sident = skon[:, K_ID:K_ID + 128]

            # Act table warm-up (one Reciprocal table load, overlaps head)
            warm = sm.tile([128, 1], f32, tag="warm")
            _act_recip(nc, mybir, warm, skon[:, 0:1])

            # PE p-state warm-up: the clock ramps for 3us from the FIRST PE
            # instruction; a dummy matmul at t~0.3 gets the stream matmuls
            # to full clock ~4us earlier than the moment phase would.
            zt = sm.tile([128, 1], f16, tag="zt")
            nc.vector.memset(zt, 1.0)
            pdum = pmisc.tile([1, 1], f32, tag="pm", name="pdum")
            nc.tensor.matmul(pdum, lhsT=zt, rhs=zt, start=True, stop=True)

            # ---- phase 1: moments (fp16 in, fp32 accum) ----
            psmom = pmisc.tile([128, 8], f32, tag="pm")
            for c in range(32):
                nc.tensor.matmul(
                    psmom[:, 0:5],
                    lhsT=pts[:, c * 128:(c + 1) * 128],
                    rhs=smbs[:, c * 5:(c + 1) * 5],
                    start=(c == 0),
                    stop=(c == 31),
                )

            # ---- phase 2: per-row quadratic-form coefficients ----
            def t(cols, tag):
                return sm.tile([128, cols], f32, tag=tag, name=tag)

            mom = t(5, "mom")
            nc.vector.tensor_copy(out=mom, in_=psmom[:, 0:5])
            pp = t(2, "pp")
            nc.vector.tensor_scalar(out=pp, in0=mom[:, 0:2], scalar1=-1.0,
                                    scalar2=EPS_DIST, op0=AT.mult, op1=AT.add)
            sq2 = t(2, "sq2")
            nc.vector.tensor_tensor(out=sq2, in0=pp, in1=pp, op=AT.mult)
            pxy = t(1, "pxy")
            nc.vector.tensor_tensor(out=pxy, in0=pp[:, 0:1], in1=pp[:, 1:2],
                                    op=AT.mult)
            musq = t(2, "musq")
            nc.vector.tensor_tensor(out=musq, in0=mom[:, 0:2], in1=mom[:, 0:2],
                                    op=AT.mult)
            mxy = t(1, "mxy")
            nc.vector.tensor_tensor(out=mxy, in0=mom[:, 0:1], in1=mom[:, 1:2],
                                    op=AT.mult)
            cov2 = t(2, "cov2")       # [cov00, cov11]
            nc.vector.tensor_tensor(out=cov2, in0=mom[:, 2:4], in1=musq,
                                    op=AT.subtract)
            cov01 = t(1, "cov01")
            nc.vector.tensor_tensor(out=cov01, in0=mom[:, 4:5], in1=mxy,
                                    op=AT.subtract)
            ra = t(1, "ra")           # 1/cov00 (+1e-12 is sub-ulp here)
            nc.vector.reciprocal_approx_fast(out=ra, in_=cov2[:, 0:1])
            c01sq = t(1, "c01sq")
            nc.vector.tensor_tensor(out=c01sq, in0=cov01, in1=cov01, op=AT.mult)
            b2 = t(1, "b2")
            nc.vector.tensor_tensor(out=b2, in0=c01sq, in1=ra, op=AT.mult)
            c2v = t(1, "c2v")         # cov11 - b2
            nc.vector.scalar_tensor_tensor(out=c2v, in0=b2, scalar=-1.0,
                                           in1=cov2[:, 1:2], op0=AT.mult,
                                           op1=AT.add)
            ac = t(1, "ac")
            nc.vector.tensor_tensor(out=ac, in0=cov2[:, 0:1], in1=c2v,
                                    op=AT.mult)
            rq = t(1, "rq")           # 1/(a2*c2)
            nc.vector.reciprocal_approx_fast(out=rq, in_=ac)
            tBC = t(1, "tBC")
            nc.vector.tensor_tensor(out=tBC, in0=b2, in1=c2v, op=AT.add)

            Q = L_INV_SCAL * L_INV_SCAL
            coef = sm.tile([128, 6], f32, tag="coef")
            nc.vector.scalar_tensor_tensor(out=coef[:, 3:4], in0=tBC, scalar=Q,
                                           in1=rq, op0=AT.mult, op1=AT.mult)
            nc.vector.scalar_tensor_tensor(out=coef[:, 4:5], in0=cov01,
                                           scalar=-2.0 * Q, in1=rq,
                                           op0=AT.mult, op1=AT.mult)
            nc.vector.scalar_tensor_tensor(out=coef[:, 5:6], in0=cov2[:, 0:1],
                                           scalar=Q, in1=rq, op0=AT.mult,
                                           op1=AT.mult)
            # c1 = 2A*py + B*px ; c2 = B*py + 2C*px   (DVE)
            t4 = t(1, "t4")
            nc.vector.scalar_tensor_tensor(out=t4, in0=coef[:, 3:4], scalar=2.0,
                                           in1=pp[:, 0:1], op0=AT.mult,
                                           op1=AT.mult)
            t5 = t(1, "t5")
            nc.vector.tensor_tensor(out=t5, in0=coef[:, 4:5], in1=pp[:, 1:2],
                                    op=AT.mult)
            nc.vector.tensor_tensor(out=coef[:, 1:2], in0=t4, in1=t5, op=AT.add)
            t6 = t(1, "t6")
            nc.vector.tensor_tensor(out=t6, in0=coef[:, 4:5], in1=pp[:, 0:1],
                                    op=AT.mult)
            t7 = t(1, "t7")
            nc.vector.scalar_tensor_tensor(out=t7, in0=coef[:, 5:6], scalar=2.0,
                                           in1=pp[:, 1:2], op0=AT.mult,
                                           op1=AT.mult)
            nc.vector.tensor_tensor(out=coef[:, 2:3], in0=t6, in1=t7, op=AT.add)
            # c0 = 1 + A*py^2 + B*py*px + C*px^2 as 3 chained Act ops
            # (Identity computes scale*x + bias with per-partition APs)
            a1 = t(1, "a1")
            nc.scalar.activation(a1, sq2[:, 0:1], mybir.ActivationFunctionType
                                 .Identity, bias=1.0, scale=coef[:, 3:4])
            a2t = t(1, "a2t")
            nc.scalar.activation(a2t, sq2[:, 1:2], mybir.ActivationFunctionType
                                 .Identity, bias=a1, scale=coef[:, 5:6])
            nc.scalar.activation(coef[:, 0:1], pxy,
                                 mybir.ActivationFunctionType.Identity,
                                 bias=a2t, scale=coef[:, 4:5])

            # coef^T as a regular fp16 matmul (lhsT=coef16, rhs=ident16),
            # written to all three partition quadrants (matmul lhsT/rhs share
            # base partition, so each basis group needs its own coefT copy).
            # Group 0 first - the stream starts on it; groups 1/2 follow
            # during the first chunks (not needed until basis col 7280).
            coef16 = sm.tile([128, 6], f16, tag="coef16")
            nc.vector.tensor_copy(out=coef16, in_=coef)
            coefTq = sm.tile([70, 128], f16, tag="coefTq")

            def rep_coef(g):
                pstq = pmisc.tile([70, 128], f32, tag="pm", name=f"pstq{g}")
                nc.tensor.matmul(pstq[32 * g:32 * g + 6, :], lhsT=coef16,
                                 rhs=sident, start=True, stop=True)
                nc.vector.tensor_copy(out=coefTq[32 * g:32 * g + 6, :],
                                      in_=pstq[32 * g:32 * g + 6, :])

            rep_coef(0)
            rep_coef(1)
            rep_coef(2)

            # ---- generation helpers ----
            def gen_mm(pg, pcol, glo, ghi):
                """Matmuls filling pg[:, pcol:...] with proj for global basis
                cols [glo, ghi), splitting at basis quadrant boundaries.
                Caller keeps each call within one 512-col PSUM bank span."""
                while glo < ghi:
                    g = glo // GRP_W
                    n = min(ghi, (g + 1) * GRP_W) - glo
                    nc.tensor.matmul(
                        pg[:, pcol:pcol + n],
                        lhsT=coefTq[32 * g:32 * g + 6, :],
                        rhs=gbs[32 * g:32 * g + 6,
                                glo - g * GRP_W:glo - g * GRP_W + n],
                        start=True, stop=True)
                    pcol += n
                    glo += n

            recip_sched = []   # "D"/"A" consumed per psum chunk

            def recip(dst, pg):
                if recip_sched.pop(0) == "D":
                    with nc.allow_low_precision(reason="fp16 heat by design"):
                        nc.vector.reciprocal(out=dst, in_=pg)
                else:
                    _act_recip(nc, mybir, dst, pg)

            def gen_chunk(glo, cols, dst, dcol):
                """One PSUM chunk [128, cols] -> reciprocal -> dst fp16."""
                pg = pgen.tile([128, cols], f32, tag="ps", name="ps")
                for q in range(0, cols, 512):
                    gen_mm(pg, q, glo + q, glo + min(q + 512, cols))
                recip(dst[:, dcol:dcol + cols], pg)

            # ---- stages 3-5: heat resident in SBUF ----
            H = {}
            recip_sched += ["D", "A", "A"]
            for sidx in (3, 4, 5):
                hw = HWS[sidx]
                Hs = sm.tile([128, hw], f16, tag=f"H{sidx}", name=f"H{sidx}")
                pg = pmisc.tile([128, hw], f32, tag="pm", name=f"pgH{sidx}")
                gen_mm(pg, 0, GB_OFF[sidx], GB_OFF[sidx] + hw)
                recip(Hs, pg)
                H[sidx] = Hs

            # part_heat outputs, stages 3-5
            nc.sync.dma_start(out=outh[:, ROFF[3]:ROFF[3] + HWS[3]], in_=H[3])
            nc.gpsimd.dma_start(out=outh[0:32, ROFF[4]:ROFF[4] + HWS[4]],
                                in_=H[4][0:32, :])
            nc.gpsimd.dma_start(out=outh[0:16, ROFF[5]:ROFF[5] + HWS[5]],
                                in_=H[5][0:16, :])

            # part-sums for the fmap normalizers (PE, ready early)
            pss = {}
            for si, sidx in enumerate((3, 4, 5)):
                ps = pmisc.tile([8, HWS[sidx]], f32, tag="pm",
                                name=f"pss{sidx}")
                nc.tensor.matmul(ps, lhsT=ssel[:, si * 8:(si + 1) * 8],
                                 rhs=H[sidx], start=True, stop=True)
                pss[sidx] = ps

            # fmap chains (emitted mid-stream so all engines stay busy)
            def fmap_chains():
                for si, sidx in enumerate((3, 4, 5)):
                    hw = HWS[sidx]
                    rr = sm.tile([8, hw], f16, tag=f"rr{sidx}",
                                 name=f"rr{sidx}")
                    _act_recip(nc, mybir, rr, pss[sidx], bias=1.0)
                    psR = pmisc.tile([128, hw], f32, tag="pm",
                                     name=f"psR{sidx}")
                    nc.tensor.matmul(psR, lhsT=srep, rhs=rr, start=True,
                                     stop=True)
                    Hn = sm.tile([128, hw], f16, tag=f"Hn{sidx}",
                                 name=f"Hn{sidx}")
                    nc.vector.tensor_tensor(out=Hn, in0=H[sidx], in1=psR,
                                            op=AT.mult)
                    fm = sm.tile([128, 4 * hw], f16, tag=f"fm{sidx}",
                                 name=f"fm{sidx}")
                    for half in range(2):
                        pf = pmisc.tile([128, 2 * hw], f32, tag="pm",
                                        name="pf")
                        for bo in range(2):
                            g = half * 2 + bo
                            nc.tensor.matmul(
                                pf[:, bo * hw:(bo + 1) * hw],
                                lhsT=swf[:, (si * 4 + g) * 128:
                                         (si * 4 + g + 1) * 128],
                                rhs=Hn, start=True, stop=True)
                        dst = fm[:, half * 2 * hw:(half + 1) * 2 * hw]
                        if half == 0:
                            nc.scalar.copy(dst, pf)
                        else:
                            nc.vector.tensor_copy(out=dst, in_=pf)
                    eng = nc.gpsimd if sidx != 4 else nc.sync
                    eng.dma_start(out=outf[:, FOFF[si]:FOFF[si] + 4 * hw],
                                  in_=fm)

            # ---- stage 0-2 streaming: 1536-col chunks, 1 DMA per chunk ----
            CH_SCHED = ["D" if i % 2 else "A" for i in range(15)]
            DMA_ENG = [0]
            hold = [fmap_chains]

            def stream_stage(sidx):
                hw = HWS[sidx]
                base = GB_OFF[sidx]
                n0 = 0
                while n0 < hw:
                    cols = min(1536, hw - n0)
                    recip_sched.append(CH_SCHED.pop(0))
                    ht = hp.tile([128, cols], f16, tag="ht", name="ht")
                    gen_chunk(base + n0, cols, ht, 0)
                    eng = (nc.sync, nc.gpsimd)[DMA_ENG[0] % 2]
                    DMA_ENG[0] += 1
                    eng.dma_start(
                        out=outh[:, ROFF[sidx] + n0:ROFF[sidx] + n0 + cols],
                        in_=ht)
                    n0 += cols
                    if hold[0] is not None and DMA_ENG[0] >= 2:
                        fn = hold[0]
                        hold[0] = None
                        fn()

            stream_stage(0)
            stream_stage(1)
            stream_stage(2)

    nc.compile()
    return nc


def _get_nc():
    if "nc" not in _NC_CACHE:
        _NC_CACHE["nc"] = _build()
    return _NC_CACHE["nc"]


def _in_maps(part_maps, features):
    part_maps = np.asarray(part_maps, dtype=np.float32)
    features = np.asarray(features, dtype=np.float32)
    gbp, konst = _host_consts()
    in_maps = []
    for core in range(NCORES):
        pm = part_maps[core * BL:(core + 1) * BL]          # [8, 16, 64, 64]
        # k-major row order: row r = k*8 + b; pt[p, c*128+r] = P[r, c*128+p]
        P = pm.transpose(1, 0, 2, 3).reshape(ROWS, HMAP * HMAP)
        pt = np.ascontiguousarray(
            P.reshape(128, 32, 128).transpose(2, 1, 0).reshape(128, 4096)
        ).astype(np.float16)
        wfc = _host_wf(features[core * BL:(core + 1) * BL])
        in_maps.append({"pt": pt, "gb": gbp, "kon": konst, "wf": wfc})
    return in_maps


def _assemble(oh, of):
    """Merge device-layout fp16 outputs into the reference layout (fp32)."""
    full = np.empty((BL, OUT_TOT), np.float32)
    fm_i = 0
    for s in range(6):
        pd, hw = STAGES[s][2], HWS[s]
        blk = oh[:pd * BL, ROFF[s]:ROFF[s] + hw].astype(np.float32)
        full[:, OUT_PH[s]:OUT_PH[s] + pd * hw] = \
            blk.reshape(pd, BL, hw).transpose(1, 0, 2).reshape(BL, pd * hw)
        if OUT_FM[s] is not None:
            fmb = of[:, FOFF[fm_i]:FOFF[fm_i] + 4 * hw].astype(np.float32)
            # fmb[bp*64+n, g*hw+f] -> full[2g+bp, n*hw+f]
            arr = fmb.reshape(2, NF, 4, hw).transpose(2, 0, 1, 3)
            full[:, OUT_FM[s]:OUT_FM[s] + NF * hw] = \
                arr.reshape(BL, NF * hw)
            fm_i += 1
    return full


def _run(part_maps, features, trace=False):
    from concourse.bass_utils import run_bass_kernel_spmd
    nc = _get_nc()
    res = run_bass_kernel_spmd(nc, _in_maps(part_maps, features),
                               list(range(NCORES)), trace=trace)
    outs = [_assemble(res.results[i]["outh"], res.results[i]["outf"])
            for i in range(NCORES)]
    return np.concatenate(outs, axis=0), res


def kernel(part_maps, features):
    out, _ = _run(part_maps, features, trace=False)
    return out
